# revision 2
# baseline (speedup 1.0000x reference)
"""Trainium2 Bass kernel for nn_BinarizeLayer (histogram_binning).

Computes, for x [524288, 32] (16 discrete cols + 16 continuous cols):
  - centers1[f,k] = i_min[f] + cumsum(max(interval[f,:], EPS))   [16,16]
  - centers2[f,k] = i_min[f] + cumsum(max(interval2[f,:], EPS))  [16,8]
  - out = concat([x_disc, onehot_argmin_d1, onehot_argmin_d2], axis=1)  [B, 400]
  - loss = sum_f mean_b sum_k d*softmax(-d)  over both center sets
where d = (cont - centers)^2.

Device algorithm (per 128-row partition-block, batch on partitions):
  e = DerivErf(cont - centers) = (2/sqrt(pi)) * exp(-d)   [ACT, fused]
  onehot = (e == max_k e)          (argmax e == argmin d)
  loss via sums:  wsum = c^2 - 2c*(S_{mu e}/S_e) + S_{mu^2 e}/S_e
  (the 2/sqrt(pi) scale cancels in the ratios)

Sharded batch-parallel over 8 NeuronCores; tiny params broadcast.
"""

import numpy as np

B = 524288
NCORES = 8
BC = B // NCORES          # rows per core
DISC = 16
F = 16
K1 = 16
K2 = 8
EPS = np.float32(0.001)
NC1 = F * K1              # 256
NC2 = F * K2              # 128
NCB = NC1 + NC2           # 384
OUTC = DISC + NCB         # 400

T = 8                     # row-blocks of 128 per super-tile
ROWS = 128 * T            # rows per super-tile
NIT = BC // ROWS          # iterations per core

_cache = {}


def _build_program():
    import concourse.bacc as bacc
    import concourse.mybir as mybir
    import concourse.tile as tile

    f32 = mybir.dt.float32
    Alu = mybir.AluOpType
    Act = mybir.ActivationFunctionType
    Ax = mybir.AxisListType

    nc = bacc.Bacc("TRN2", target_bir_lowering=False, debug=False,
                   enable_asserts=False, num_devices=1)
    x_d = nc.dram_tensor("x", [BC, 32], f32, kind="ExternalInput").ap()
    cb_d = nc.dram_tensor("cb", [128, NCB], f32, kind="ExternalInput").ap()
    y_d = nc.dram_tensor("y", [BC, OUTC], f32, kind="ExternalOutput").ap()
    lp_d = nc.dram_tensor("lossp", [128, 1], f32, kind="ExternalOutput").ap()

    with tile.TileContext(nc) as tc:
        with tc.tile_pool(name="const", bufs=1) as cpool, \
             tc.tile_pool(name="big", bufs=2) as bpool, \
             tc.tile_pool(name="small", bufs=2) as spool:
            cb = cpool.tile([128, NCB], f32)
            nc.sync.dma_start(out=cb[:], in_=cb_d[:, :])
            acc = cpool.tile([128, T * 32], f32)
            nc.vector.memset(acc[:], 0.0)

            # broadcast views of the centers
            cb_t = cb[:].unsqueeze(1).broadcast_to([128, T, NCB])
            cb1 = cb[:, 0:NC1].rearrange("p (f k) -> p f k", k=K1) \
                .unsqueeze(1).broadcast_to([128, T, F, K1])
            cb2 = cb[:, NC1:NCB].rearrange("p (f k) -> p f k", k=K2) \
                .unsqueeze(1).broadcast_to([128, T, F, K2])

            for it in range(NIT):
                r0 = it * ROWS
                X = bpool.tile([128, T * 32], f32)
                nc.sync.dma_start(
                    out=X[:].rearrange("p (t j) -> p t j", t=T),
                    in_=x_d[r0:r0 + ROWS, :].rearrange("(t p) j -> p t j", p=128),
                )
                Xv = X[:].rearrange("p (t j) -> p t j", t=T)
                c1v = Xv[:, :, 16:32].unsqueeze(3).broadcast_to([128, T, F, K1])
                c2v = Xv[:, :, 16:32].unsqueeze(3).broadcast_to([128, T, F, K2])

                E = bpool.tile([128, T * NCB], f32)
                Ev = E[:].rearrange("p (t c) -> p t c", t=T)
                E1 = Ev[:, :, 0:NC1].rearrange("p t (f k) -> p t f k", k=K1)
                E2 = Ev[:, :, NC1:NCB].rearrange("p t (f k) -> p t f k", k=K2)
                nc.vector.tensor_tensor(E1, c1v, cb1, Alu.subtract)
                nc.vector.tensor_tensor(E2, c2v, cb2, Alu.subtract)
                # e = (2/sqrt(pi)) exp(-(c-mu)^2)
                nc.scalar.activation(E[:], E[:], Act.Derivative_Erf)

                OH = bpool.tile([128, T * OUTC], f32)
                OHv = OH[:].rearrange("p (t c) -> p t c", t=T)
                nc.scalar.copy(OHv[:, :, 0:DISC], Xv[:, :, 0:DISC])

                m = spool.tile([128, T * 32], f32)
                mv = m[:].rearrange("p (t j) -> p t j", t=T)
                nc.vector.tensor_reduce(mv[:, :, 0:16], E1, Ax.X, Alu.max)
                nc.vector.tensor_reduce(mv[:, :, 16:32], E2, Ax.X, Alu.max)
                m1b = mv[:, :, 0:16].unsqueeze(3).broadcast_to([128, T, F, K1])
                m2b = mv[:, :, 16:32].unsqueeze(3).broadcast_to([128, T, F, K2])
                O1 = OHv[:, :, DISC:DISC + NC1].rearrange(
                    "p t (f k) -> p t f k", k=K1)
                O2 = OHv[:, :, DISC + NC1:OUTC].rearrange(
                    "p t (f k) -> p t f k", k=K2)
                nc.vector.tensor_tensor(O1, E1, m1b, Alu.is_equal)
                nc.vector.tensor_tensor(O2, E2, m2b, Alu.is_equal)

                S = spool.tile([128, T * 96], f32)
                Sv = S[:].rearrange("p (t j) -> p t j", t=T)
                nc.vector.tensor_reduce(Sv[:, :, 0:16], E1, Ax.X, Alu.add)
                nc.vector.tensor_reduce(Sv[:, :, 16:32], E2, Ax.X, Alu.add)

                M = bpool.tile([128, T * NCB], f32)
                Mv = M[:].rearrange("p (t c) -> p t c", t=T)
                M1 = Mv[:, :, 0:NC1].rearrange("p t (f k) -> p t f k", k=K1)
                M2 = Mv[:, :, NC1:NCB].rearrange("p t (f k) -> p t f k", k=K2)
                nc.vector.tensor_tensor(Mv, Ev, cb_t, Alu.mult)      # mu*e
                nc.vector.tensor_reduce(Sv[:, :, 32:48], M1, Ax.X, Alu.add)
                nc.vector.tensor_reduce(Sv[:, :, 48:64], M2, Ax.X, Alu.add)
                nc.vector.tensor_tensor(Mv, Mv, cb_t, Alu.mult)      # mu^2*e
                nc.vector.tensor_reduce(Sv[:, :, 64:80], M1, Ax.X, Alu.add)
                nc.vector.tensor_reduce(Sv[:, :, 80:96], M2, Ax.X, Alu.add)

                nc.sync.dma_start(
                    out=y_d[r0:r0 + ROWS, :].rearrange("(t p) c -> p t c", p=128),
                    in_=OHv,
                )

                r = spool.tile([128, T * 32], f32)
                rv = r[:].rearrange("p (t j) -> p t j", t=T)
                nc.vector.reciprocal(rv, Sv[:, :, 0:32])
                R1 = spool.tile([128, T * 32], f32)
                R1v = R1[:].rearrange("p (t j) -> p t j", t=T)
                nc.vector.tensor_tensor(R1v, Sv[:, :, 32:64], rv, Alu.mult)
                R2 = spool.tile([128, T * 32], f32)
                R2v = R2[:].rearrange("p (t j) -> p t j", t=T)
                nc.vector.tensor_tensor(R2v, Sv[:, :, 64:96], rv, Alu.mult)

                u = spool.tile([128, T * 32], f32)
                uv = u[:].rearrange("p (t j) -> p t j", t=T)
                nc.vector.tensor_tensor(uv[:, :, 0:16], Xv[:, :, 16:32],
                                        R1v[:, :, 0:16], Alu.subtract)
                nc.vector.tensor_tensor(uv[:, :, 16:32], Xv[:, :, 16:32],
                                        R1v[:, :, 16:32], Alu.subtract)
                # w = u^2 + (R2 - R1^2); acc += w
                w = spool.tile([128, T * 32], f32)
                nc.vector.tensor_tensor(w[:], u[:], u[:], Alu.mult)
                nc.vector.tensor_tensor(R1[:], R1[:], R1[:], Alu.mult)
                nc.vector.tensor_tensor(R2[:], R2[:], R1[:], Alu.subtract)
                nc.vector.tensor_tensor(w[:], w[:], R2[:], Alu.add)
                nc.vector.tensor_tensor(acc[:], acc[:], w[:], Alu.add)

            lp = cpool.tile([128, 1], f32)
            nc.vector.tensor_reduce(lp[:], acc[:], Ax.X, Alu.add)
            nc.sync.dma_start(out=lp_d[:, :], in_=lp[:])

    nc.compile()
    return nc


def _get_program():
    if "nc" not in _cache:
        _cache["nc"] = _build_program()
    return _cache["nc"]


def _centers(interval, i_min):
    g = np.maximum(interval.astype(np.float32), EPS)
    return (i_min.astype(np.float32)[:, None]
            + np.cumsum(g, axis=1, dtype=np.float32))


def kernel(x, interval, interval2, i_min):
    import os
    from concourse.bass_utils import run_bass_kernel_spmd

    x = np.ascontiguousarray(np.asarray(x, dtype=np.float32))
    c1 = _centers(np.asarray(interval), np.asarray(i_min))     # [16,16]
    c2 = _centers(np.asarray(interval2), np.asarray(i_min))    # [16,8]
    cb_row = np.concatenate([c1.reshape(-1), c2.reshape(-1)])  # [384]
    cb = np.ascontiguousarray(
        np.broadcast_to(cb_row[None, :], (128, NCB)).astype(np.float32))

    nc = _get_program()
    in_maps = [{"x": x[c * BC:(c + 1) * BC], "cb": cb} for c in range(NCORES)]

    trace = bool(int(os.environ.get("BINARIZE_TRACE", "0")))
    if trace:
        try:
            import sys, types
            import antenv
            if "antenv.axon_hooks" not in sys.modules:
                from trn_agent_boot.trn_boot import _ntff_profile_via_ctypes
                hook = _ntff_profile_via_ctypes("/opt/axon/libaxon_pjrt.so")
                mod = types.ModuleType("antenv.axon_hooks")
                mod._hook = hook
                mod.get_axon_ntff_profile_hook = lambda: mod._hook
                mod.set_axon_ntff_profile_hook = (
                    lambda h: setattr(mod, "_hook", h))
                antenv.axon_hooks = mod
                sys.modules["antenv.axon_hooks"] = mod
        except Exception:
            trace = False

    res = run_bass_kernel_spmd(nc, in_maps, core_ids=list(range(NCORES)),
                               trace=trace)
    _cache["last_result"] = res

    y = np.concatenate([res.results[c]["y"] for c in range(NCORES)], axis=0)
    lsum = sum(float(res.results[c]["lossp"].astype(np.float64).sum())
               for c in range(NCORES))
    loss = np.float32(lsum / B)
    return y, loss
